# revision 60
# baseline (speedup 1.0000x reference)
"""Distributed 2-layer GAT on 8 Trainium2 NeuronCores (Bass/Tile).

Strategy (graph/data parallel):
  - Nodes sharded across 8 cores (6250 each, padded to 6272 = 49*128);
    within a core, nodes are sorted by in-degree (desc) into 49 tiles of
    128 so per-tile edge counts match across cores; pads sit in the last
    tile's tail rows.
  - All compute tables in bf16; layer tables ([h | al_src], channel-major
    head layout, rows padded to a 256B multiple for dma_gather) computed
    locally (x @ [W1|W1a_s|W1a_d] extended matmul) and AllGathered so every
    core holds the full node table in DRAM.
  - Edge pass per destination tile: edges (excluding the appended self
    loops) are packed contiguously into 128-slot columns, sorted by source
    row and split into a low/high half-table block (dma_gather indices are
    int16, so each gather addresses < 32768 rows). Per tile: ONE batched
    dma_gather per half fetches all source rows (the 994ns SWDGE fixed
    cost is paid per tile, not per column). Per column: a one-hot
    selection matrix oh[slot, dst] built by a single tensor_scalar
    is_equal against an iota constant; al_dst per edge via PE-transpose of
    oh and a small matmul against the tile's al_dst vector;
    ex = exp(leaky_relu(al_src+al_dst)) overwrites the gathered al_src
    columns so one accumulating matmul per column scatters both the
    ex-weighted rows and the softmax denominators into PSUM.
  - Self loops never touch DRAM: their contribution is computed from the
    resident per-tile h rows and added on the vector engine.
  - Layer-2 local matmul is fused into the layer-1 edge loop so the second
    AllGather starts as soon as the last tile finishes.
"""

import os
import sys
import types

import ml_dtypes
import numpy as np

_BUILD_CACHE = {}
BF = ml_dtypes.bfloat16
F8 = ml_dtypes.float8_e4m3


def _register_trace_hook():
    try:
        if "antenv.axon_hooks" in sys.modules:
            return True
        from trn_agent_boot.trn_boot import _ntff_profile_via_ctypes

        hook = _ntff_profile_via_ctypes("/opt/axon/libaxon_pjrt.so")
        m = types.ModuleType("antenv.axon_hooks")
        m.get_axon_ntff_profile_hook = lambda: hook
        m.set_axon_ntff_profile_hook = lambda h: None
        sys.modules["antenv.axon_hooks"] = m
        return True
    except Exception:
        return False


def _host_prep(x, edge_index, W1, a_src1, a_dst1, b1, W2, a_src2, a_dst2, b2, C):
    x = np.asarray(x, np.float32)
    ei = np.asarray(edge_index)
    W1 = np.asarray(W1, np.float32)
    a_src1 = np.asarray(a_src1, np.float32)
    a_dst1 = np.asarray(a_dst1, np.float32)
    b1 = np.asarray(b1, np.float32)
    W2 = np.asarray(W2, np.float32)
    a_src2 = np.asarray(a_src2, np.float32)
    a_dst2 = np.asarray(a_dst2, np.float32)
    b2 = np.asarray(b2, np.float32)

    N, F = x.shape
    HEADS, HID = a_src1.shape
    D1 = HEADS * HID
    NCLS = W2.shape[1]
    assert N % C == 0
    NPC = N // C
    NT = -(-NPC // 128)
    PADN = NT * 128
    assert PADN > NPC, "need at least one pad slot per core"
    NPALL = C * PADN
    E = ei.shape[1]
    HB = 32768
    RW1 = 384  # [h(256) | al_src(4) | pad] -> 768B rows
    RW2 = 128  # [z2(64) | al_src2(1) | pad] -> 256B rows

    src = ei[0].astype(np.int64)
    dst = ei[1].astype(np.int64)

    # --- per-core in-degree sort into tiles (pads land in last tile) ---
    deg = np.bincount(dst, minlength=N).astype(np.int64)
    pos = np.empty(N, np.int64)
    for c in range(C):
        lo = c * NPC
        order = np.argsort(-deg[lo:lo + NPC], kind="stable")
        p = np.empty(NPC, np.int64)
        p[order] = np.arange(NPC)
        pos[lo:lo + NPC] = p

    ncidx = np.arange(N) // NPC
    node_at = np.full((C, PADN), -1, np.int64)
    node_at[ncidx, pos] = np.arange(N)
    # chunk-major table rows: per chunk, all cores' rows are contiguous.
    # chunk boundary 1 is aligned with the int16 half-table split (HB) so
    # low-half gathers depend only on the first two AllGather chunks.
    NCH = 3
    mid = HB // C
    q1 = (mid // 2 // 128) * 128
    bnds = [0, q1, mid, PADN]
    grow = np.zeros(N, np.int64)
    base = 0
    cbase_h = [0]
    for ci in range(NCH):
        lo, hi = bnds[ci], bnds[ci + 1]
        m = (pos >= lo) & (pos < hi)
        grow[m] = base + ncidx[m] * (hi - lo) + (pos[m] - lo)
        base += C * (hi - lo)
        cbase_h.append(base)
    R1 = bnds[1]

    # --- per-(core, tile) slot packing, sorted by source row, split at HB ---
    ec = dst // NPC
    et = pos[dst] // 128
    gkey = ec * NT + et
    order_e = np.lexsort((grow[src], gkey))
    ks = gkey[order_e]
    s_src = src[order_e]
    s_dst = dst[order_e]
    glo = grow[s_src]
    gstart = np.searchsorted(ks, np.arange(C * NT))
    gend = np.searchsorted(ks, np.arange(C * NT) + 1)
    nlo = np.array([
        np.searchsorted(glo[gstart[g]:gend[g]], HB) for g in range(C * NT)
    ], dtype=np.int64)
    nhi = (gend - gstart) - nlo
    KLO = (-(-nlo.reshape(C, NT) // 128)).max(axis=0)
    KHI = (-(-nhi.reshape(C, NT) // 128)).max(axis=0)
    ktile = KLO + KHI
    koff = np.concatenate([[0], np.cumsum(ktile)]).astype(np.int64)
    SC = int(koff[-1])

    srcg = np.zeros((C, 128, SC), np.int16)  # pad slots read row 0 (oh col is 0)
    rank = np.arange(E) - gstart[ks]
    lo_e = rank < nlo[ks]
    rr = np.where(lo_e, rank, rank - nlo[ks])
    cc_e = ks // NT
    tt_e = ks % NT
    cbase_e = np.where(lo_e, koff[tt_e], koff[tt_e] + KLO[tt_e])
    col_e = cbase_e + rr // 128
    pp_e = rr % 128
    srcg[cc_e, pp_e, col_e] = np.where(lo_e, glo, glo - HB).astype(np.int16)
    dl_e = (pos[s_dst] % 128).astype(np.int64)

    # host-precomputed one-hot selection matrices (slot-major, fp8 — exact
    # 0/1, half the DMA bytes, 2x PE rate):
    #   oh[p, col*128 + q] = 1 iff slot p of col has dest lane q
    # (the dest-major transpose is derived on-chip via PE transpose)
    ohdir = np.zeros((C, 128, SC * 128), F8)
    ohdir[cc_e, pp_e, col_e * 128 + dl_e] = 1

    # wrapped int16 idx layout for dma_gather: within each gather call the
    # i-th index (i = col_local*128 + p) lives at [i % 16, i // 16]; the
    # 16-partition block is replicated across the 8 gpsimd core groups.
    sg = srcg.reshape(C, 8, 16, SC).transpose(0, 2, 3, 1).reshape(C, 16, SC * 8)
    sg16 = np.tile(sg, (1, 8, 1)).copy()

    # --- per-core transposed x shards (pad rows zero), bf16, pre-swizzled so
    # each tile's lhsT loads as one contiguous run per partition ---
    xs = np.zeros((C, PADN, F), np.float32)
    xs[ncidx, pos] = x
    xsT = np.ascontiguousarray(xs.transpose(0, 2, 1)).astype(BF)
    FK_ = F // 128
    xswz = np.ascontiguousarray(
        xsT.reshape(C, FK_, 128, PADN // 128, 128).transpose(0, 2, 3, 1, 4)
    ).reshape(C, 128, (PADN // 128) * FK_ * 128)

    # --- extended weights, channel-major head layout: col j = c*H + h ---
    perm = (np.arange(D1).reshape(HEADS, HID).T).reshape(-1)  # cm -> orig
    W1cm = W1[:, perm]
    Wa_s1 = np.einsum("fhc,hc->fh", W1.reshape(F, HEADS, HID), a_src1)
    Wa_d1 = np.einsum("fhc,hc->fh", W1.reshape(F, HEADS, HID), a_dst1)
    W1e = np.concatenate([W1cm, Wa_s1, Wa_d1], axis=1).astype(BF)
    W2cm = W2[perm, :]
    Wa_s2 = W2 @ a_src2[0]
    Wa_d2 = W2 @ a_dst2[0]
    W2e = np.concatenate(
        [W2cm, Wa_s2[perm, None], Wa_d2[perm, None]], axis=1
    ).astype(BF)
    b1r = np.broadcast_to(b1[perm][None, :], (128, D1)).astype(BF).copy()
    b2r = np.broadcast_to(b2[None, :], (128, NCLS)).astype(np.float32).copy()
    ident = np.eye(128, dtype=np.float32).astype(BF)
    ones = np.ones((128, D1), np.float32).astype(BF)
    ident8 = np.eye(128, dtype=np.float32).astype(F8)
    padneg = np.zeros((128, 1), np.float32)
    lastrow = NPC - (NT - 1) * 128
    padneg[lastrow:, 0] = -15.0

    cfg = dict(F=F, HEADS=HEADS, HID=HID, D1=D1, NCLS=NCLS, NT=NT, PADN=PADN, R1=R1,
               NPC=NPC, NPALL=NPALL, SC=SC, C=C, HB=HB, RW1=RW1, RW2=RW2,
               ktile=tuple(int(k) for k in ktile),
               klo=tuple(int(k) for k in KLO))
    in_maps = []
    for c in range(C):
        in_maps.append({
            "xswz": xswz[c],
            "w1e": W1e,
            "w2e": W2e,
            "b1r": b1r,
            "b2r": b2r,
            "ident": ident,
            "ones": ones,
            "ident8": ident8,
            "sg16": sg16[c],
            "ohdir": ohdir[c],
            "padneg": padneg,
        })
    return cfg, in_maps, node_at, (N, NCLS)


def _build_program(F, HEADS, HID, D1, NCLS, NT, PADN, R1, NPC, NPALL, SC, C, HB,
                   RW1, RW2, ktile, klo):
    import concourse.bacc as bacc
    import concourse.bass as bass
    import concourse.mybir as mybir
    import concourse.tile as tile

    f32 = mybir.dt.float32
    bf16 = mybir.dt.bfloat16
    f8 = mybir.dt.float8e4
    i16 = mybir.dt.int16
    AF = mybir.ActivationFunctionType
    ALU = mybir.AluOpType
    AX = mybir.AxisListType

    H = HEADS
    TW1 = D1 + 2 * H              # [h | al_src | al_dst]
    G1W = D1 + H                  # layer-1 table row payload [h | al_src]
    TW2 = NCLS + 2                # [z2 | al_src2 | al_dst2]
    G2W = NCLS + 1                # layer-2 table row payload [z2 | al_src2]
    FK = F // 128
    DK = D1 // 128
    koff = [0]
    for k in ktile:
        koff.append(koff[-1] + k)

    nc = bacc.Bacc("TRN2", target_bir_lowering=False, debug=False, num_devices=C,
                   num_swdge_queues=4)

    xsw = nc.dram_tensor("xswz", [128, NT * (F // 128) * 128], bf16,
                         kind="ExternalInput")
    w1e = nc.dram_tensor("w1e", [F, TW1], bf16, kind="ExternalInput")
    w2e = nc.dram_tensor("w2e", [D1, TW2], bf16, kind="ExternalInput")
    b1r = nc.dram_tensor("b1r", [128, D1], bf16, kind="ExternalInput")
    b2r = nc.dram_tensor("b2r", [128, NCLS], f32, kind="ExternalInput")
    idn = nc.dram_tensor("ident", [128, 128], bf16, kind="ExternalInput")
    onesd = nc.dram_tensor("ones", [128, D1], bf16, kind="ExternalInput")
    idn8 = nc.dram_tensor("ident8", [128, 128], f8, kind="ExternalInput")
    sgd = nc.dram_tensor("sg16", [128, SC * 8], i16, kind="ExternalInput")
    ohd = nc.dram_tensor("ohdir", [128, SC * 128], f8, kind="ExternalInput")
    pdn = nc.dram_tensor("padneg", [128, 1], f32, kind="ExternalInput")
    outp = nc.dram_tensor("outp", [PADN, NCLS], f32, kind="ExternalOutput")

    loc1 = nc.dram_tensor("loc1", [PADN, RW1], bf16)
    tab1 = nc.dram_tensor("tab1", [NPALL, RW1], bf16, addr_space="Shared")
    loc2 = nc.dram_tensor("loc2", [PADN, RW2], bf16)
    tab2 = nc.dram_tensor("tab2", [NPALL, RW2], bf16, addr_space="Shared")

    rg = [list(range(C))]

    with tile.TileContext(nc) as tc:
        with (
            tc.tile_pool(name="const", bufs=1) as const,
            tc.tile_pool(name="ha", bufs=1) as hap,
            tc.tile_pool(name="hd", bufs=1) as hdp,
            tc.tile_pool(name="wk", bufs=2) as wk,
            tc.tile_pool(name="g", bufs=3) as gp,
            tc.tile_pool(name="g2", bufs=8) as gp2,
            tc.tile_pool(name="oh", bufs=4) as ohp,
            tc.tile_pool(name="ohT", bufs=3) as ohtp,
            tc.tile_pool(name="ps", bufs=3, space="PSUM") as psp,
            tc.tile_pool(name="psmd", bufs=1, space="PSUM") as psmd,
            tc.tile_pool(name="pstr", bufs=1, space="PSUM") as pstr,
            tc.tile_pool(name="pstr8", bufs=2, space="PSUM") as pstr8,
            tc.tile_pool(name="psed", bufs=2, space="PSUM") as psed,
        ):
            # ---- constants ----
            w1t = []
            for kk in range(FK):
                t_ = const.tile([128, TW1], bf16, tag=f"w1_{kk}")
                nc.sync.dma_start(out=t_[:], in_=w1e[kk * 128:(kk + 1) * 128, :])
                w1t.append(t_)
            w2t = []
            for kk in range(DK):
                t_ = const.tile([128, TW2], bf16, tag=f"w2_{kk}")
                nc.sync.dma_start(out=t_[:], in_=w2e[kk * 128:(kk + 1) * 128, :])
                w2t.append(t_)
            b1s = const.tile([128, D1], bf16, tag="b1")
            nc.sync.dma_start(out=b1s[:], in_=b1r[:, :])
            b2s = const.tile([128, NCLS], f32, tag="b2")
            nc.sync.dma_start(out=b2s[:], in_=b2r[:, :])
            ids = const.tile([128, 128], bf16, tag="ident")
            nc.sync.dma_start(out=ids[:], in_=idn[:, :])
            on1 = const.tile([128, D1], bf16, tag="ones")
            nc.sync.dma_start(out=on1[:], in_=onesd[:, :])
            ids8 = const.tile([128, 128], f8, tag="ident8")
            nc.sync.dma_start(out=ids8[:], in_=idn8[:, :])
            sgall = const.tile([128, SC * 8], i16, tag="sgall")
            nc.sync.dma_start(out=sgall[:], in_=sgd[:, :])
            pds = const.tile([128, 1], f32, tag="padneg")
            nc.sync.dma_start(out=pds[:], in_=pdn[:, :])
            ssum = const.tile([128, NT], f32, tag="ssum")
            lgs = const.tile([128, NT], f32, tag="lgs")

            ha_tiles = []
            NCH = 3
            mid = HB // C
            q1 = (mid // 2 // 128) * 128
            bnds = [0, q1, mid, PADN]
            cbase = [0]
            for ci in range(NCH):
                cbase.append(cbase[-1] + C * (bnds[ci + 1] - bnds[ci]))
            # ---- phase A: local h = x @ [W1 | Wa_src | Wa_dst] ----
            with nc.named_scope("l1_local_mm"):
                for t in range(NT):
                    xt = wk.tile([128, FK * 128], bf16, tag="xt")
                    nc.sync.dma_start(
                        out=xt[:],
                        in_=xsw[:, t * FK * 128:(t + 1) * FK * 128])
                    ps_a = psp.tile([128, TW1], f32, tag="mm")
                    for kk in range(FK):
                        nc.tensor.matmul(ps_a[:], lhsT=xt[:, kk * 128:(kk + 1) * 128],
                                         rhs=w1t[kk][:],
                                         start=(kk == 0), stop=(kk == FK - 1))
                    ha = hap.tile([128, TW1], bf16, tag=f"ha_{t}")
                    nc.scalar.copy(ha[:], ps_a[:])
                    if t == NT - 1:
                        nc.vector.tensor_scalar(ha[:, D1:D1 + H],
                                                ha[:, D1:D1 + H],
                                                pds[:, 0:1], None, op0=ALU.add)
                    nc.sync.dma_start(out=loc1[t * 128:(t + 1) * 128, 0:G1W],
                                      in_=ha[:, 0:G1W])
                    ha_tiles.append(ha)
                    for ci in range(NCH - 1):
                        if bnds[ci + 1] > bnds[ci] and t == bnds[ci + 1] // 128 - 1:
                            with nc.named_scope(f"l1_allgather_{ci}"):
                                nc.gpsimd.collective_compute(
                                    "AllGather", mybir.AluOpType.bypass,
                                    replica_groups=rg,
                                    ins=[loc1[bnds[ci]:bnds[ci + 1], :]],
                                    outs=[tab1[cbase[ci]:cbase[ci + 1], :]],
                                )

            # ---- phase B: allgather layer-1 table (tail chunk) ----
            with nc.named_scope("l1_allgather"):
                nc.gpsimd.collective_compute(
                    "AllGather", mybir.AluOpType.bypass, replica_groups=rg,
                    ins=[loc1[bnds[NCH - 1]:PADN, :]],
                    outs=[tab1[cbase[NCH - 1]:NPALL, :]],
                )

            # ---- phase C: layer-1 edge pass (+ fused layer-2 local mm) ----
            hd_tiles = []
            with nc.named_scope("l1_edges"):
                def front1(t):
                    K = ktile[t]
                    kl = klo[t]
                    kh = K - kl
                    ha = ha_tiles[t]
                    g = gp.tile([128, max(K, 1) * RW1], bf16, tag="g1")
                    gv = g[:].rearrange("p (k c) -> p k c", c=RW1)
                    qn = t % 4
                    # the dma_gather ucode crashes above 1024 idxs per call
                    for a in range(0, kl, 8):
                        b = min(a + 8, kl)
                        nc.gpsimd.dma_gather(
                            out_ap=gv[:, a:b, :],
                            in_ap=tab1[0:HB, :],
                            idxs_ap=sgall[:, (koff[t] + a) * 8:(koff[t] + b) * 8],
                            num_idxs=(b - a) * 128,
                            num_idxs_reg=(b - a) * 128,
                            elem_size=RW1,
                            queue_num=qn,
                        )
                        qn = (qn + 1) % 4
                    for a in range(kl, K, 8):
                        b = min(a + 8, K)
                        nc.gpsimd.dma_gather(
                            out_ap=gv[:, a:b, :],
                            in_ap=tab1[HB:NPALL, :],
                            idxs_ap=sgall[:, (koff[t] + a) * 8:(koff[t] + b) * 8],
                            num_idxs=(b - a) * 128,
                            num_idxs_reg=(b - a) * 128,
                            elem_size=RW1,
                            queue_num=qn,
                        )
                        qn = (qn + 1) % 4
                    oh = ohp.tile([128, max(K, 1) * 128], f8, tag="oh")
                    nc.sync.dma_start(
                        out=oh[:],
                        in_=ohd[:, koff[t] * 128:(koff[t] + max(K, 1)) * 128])
                    ps_e = psed.tile([128, max(K, 1) * H], f32, tag="ed")
                    for j in range(K):
                        psT = pstr8.tile([128, 128], f8, tag="tr8")
                        nc.tensor.transpose(psT[:], oh[:, j * 128:(j + 1) * 128],
                                            ids8[:])
                        ohT = ohtp.tile([128, 128], f8, tag="ohT")
                        nc.scalar.copy(ohT[:], psT[:])
                        nc.tensor.matmul(ps_e[:, j * H:(j + 1) * H], lhsT=ohT[:],
                                         rhs=ha[:, D1 + H:D1 + 2 * H],
                                         start=True, stop=True)
                    return g, gv, oh, ps_e

                def back1(t, st):
                    K = ktile[t]
                    ha = ha_tiles[t]
                    g, gv, oh, ps_e = st
                    if K > 0:
                        ev = wk.tile([128, K * H], bf16, tag="ev")
                        nc.vector.tensor_add(
                            ev[:].rearrange("p (k h) -> p k h", h=H),
                            gv[:, :, D1:D1 + H],
                            ps_e[:].rearrange("p (k h) -> p k h", h=H))
                        lr = wk.tile([128, K * H], bf16, tag="lr")
                        nc.scalar.activation(lr[:], ev[:], AF.Prelu, alpha=0.2)
                        nc.scalar.activation(
                            gv[:, :, D1:D1 + H],
                            lr[:].rearrange("p (k h) -> p k h", h=H), AF.Exp)
                        gh = gv[:, :, 0:D1].rearrange("p k (c h) -> p k c h", h=H)
                        exb = gv[:, :, D1:D1 + H].unsqueeze(2).to_broadcast(
                            [128, K, HID, H])
                        nc.vector.tensor_mul(gh, gh, exb)
                    ps_c = psp.tile([128, G1W], f32, tag="mm")
                    if K == 0:
                        nc.vector.memset(ps_c[:], 0.0)
                    for j in range(K):
                        nc.tensor.matmul(ps_c[:], lhsT=oh[:, j * 128:(j + 1) * 128],
                                         rhs=g[:, j * RW1:j * RW1 + G1W],
                                         start=(j == 0), stop=(j == K - 1))
                    es = wk.tile([128, H], bf16, tag="es")
                    nc.vector.tensor_add(es[:], ha[:, D1:D1 + H],
                                         ha[:, D1 + H:D1 + 2 * H])
                    lrs = wk.tile([128, H], bf16, tag="lrs")
                    nc.scalar.activation(lrs[:], es[:], AF.Prelu, alpha=0.2)
                    exs1 = wk.tile([128, H], bf16, tag="exs1")
                    nc.scalar.activation(exs1[:], lrs[:], AF.Exp)
                    selfc = wk.tile([128, D1], bf16, tag="selfc")
                    nc.vector.tensor_mul(
                        selfc[:].rearrange("p (c h) -> p c h", h=H),
                        ha[:, 0:D1].rearrange("p (c h) -> p c h", h=H),
                        exs1[:].unsqueeze(1).to_broadcast([128, HID, H]))
                    den = wk.tile([128, H], f32, tag="den")
                    nc.vector.tensor_add(den[:], ps_c[:, D1:D1 + H], exs1[:])
                    rec = wk.tile([128, H], f32, tag="rec")
                    nc.vector.reciprocal(rec[:], den[:])
                    o1n = wk.tile([128, D1], f32, tag="o1n")
                    nc.vector.tensor_add(o1n[:], ps_c[:, 0:D1], selfc[:])
                    o1 = wk.tile([128, D1], bf16, tag="o1")
                    nc.vector.tensor_mul(
                        o1[:].rearrange("p (c h) -> p c h", h=H),
                        o1n[:].rearrange("p (c h) -> p c h", h=H),
                        rec[:].unsqueeze(1).to_broadcast([128, HID, H]))
                    nc.vector.tensor_add(o1[:], o1[:], b1s[:])
                    rl = wk.tile([128, D1], bf16, tag="rl")
                    nc.scalar.activation(rl[:], o1[:], AF.Relu)
                    tn = wk.tile([128, D1], bf16, tag="tn")
                    nc.vector.tensor_sub(tn[:], o1[:], rl[:])
                    nc.scalar.activation(tn[:], tn[:], AF.Exp)
                    nc.vector.tensor_add(o1[:], rl[:], tn[:])
                    nc.vector.tensor_sub(o1[:], o1[:], on1[:])
                    tts = []
                    for kk in range(DK):
                        ps_t = pstr.tile([128, 128], bf16, tag="tr")
                        nc.tensor.transpose(ps_t[:], o1[:, kk * 128:(kk + 1) * 128],
                                            ids[:])
                        tt = wk.tile([128, 128], bf16, tag=f"tt{kk}")
                        nc.scalar.copy(tt[:], ps_t[:])
                        tts.append(tt)
                    ps_d = psmd.tile([128, TW2], f32, tag="md")
                    for kk in range(DK):
                        nc.tensor.matmul(ps_d[:], lhsT=tts[kk][:], rhs=w2t[kk][:],
                                         start=(kk == 0), stop=(kk == DK - 1))
                    hd = hdp.tile([128, TW2], bf16, tag=f"hd_{t}")
                    nc.scalar.copy(hd[:], ps_d[:])
                    if t == NT - 1:
                        nc.vector.tensor_scalar(hd[:, NCLS:NCLS + 1],
                                                hd[:, NCLS:NCLS + 1],
                                                pds[:, 0:1], None, op0=ALU.add)
                    nc.sync.dma_start(out=loc2[t * 128:(t + 1) * 128, 0:G2W],
                                      in_=hd[:, 0:G2W])
                    hd_tiles.append(hd)
                    for ci in range(NCH - 1):
                        if bnds[ci + 1] > bnds[ci] and t == bnds[ci + 1] // 128 - 1:
                            with nc.named_scope(f"l2_allgather_{ci}"):
                                nc.gpsimd.collective_compute(
                                    "AllGather", mybir.AluOpType.bypass,
                                    replica_groups=rg,
                                    ins=[loc2[bnds[ci]:bnds[ci + 1], :]],
                                    outs=[tab2[cbase[ci]:cbase[ci + 1], :]],
                                )

                # prefetch the first l2 tiles' low-half gathers into the
                # l1 tail where gpsimd is idle (low half of tab2 is complete
                # once the chunk-0 l2 allgather, issued at tile 31, lands).
                pre2 = {}

                def front2lo(t2):
                    K = ktile[t2]
                    kl = klo[t2]
                    g = gp2.tile([128, max(K, 1) * RW2], bf16, tag="g2")
                    gv = g[:].rearrange("p (k c) -> p k c", c=RW2)
                    qn = t2 % 4
                    for a in range(0, kl, 8):
                        b = min(a + 8, kl)
                        nc.gpsimd.dma_gather(
                            out_ap=gv[:, a:b, :],
                            in_ap=tab2[0:HB, :],
                            idxs_ap=sgall[:, (koff[t2] + a) * 8:(koff[t2] + b) * 8],
                            num_idxs=(b - a) * 128,
                            num_idxs_reg=(b - a) * 128,
                            elem_size=RW2,
                            queue_num=qn,
                        )
                        qn = (qn + 1) % 4
                    return g, gv

                NPRE = 7
                stc = {}
                for t in range(NT):
                    stc[t] = front1(t)
                    if t >= 1:
                        back1(t - 1, stc.pop(t - 1))
                        pt = (t - 1) - (NT - 1 - NPRE)
                        if 0 <= pt < NPRE:
                            pre2[pt] = front2lo(pt)
                back1(NT - 1, stc.pop(NT - 1))

            # ---- phase E: allgather layer-2 table (tail chunk) ----
            with nc.named_scope("l2_allgather"):
                nc.gpsimd.collective_compute(
                    "AllGather", mybir.AluOpType.bypass, replica_groups=rg,
                    ins=[loc2[bnds[NCH - 1]:PADN, :]],
                    outs=[tab2[cbase[NCH - 1]:NPALL, :]],
                )

            # ---- phase F: layer-2 edge pass ----
            with nc.named_scope("l2_edges"):
                def front2(t):
                    K = ktile[t]
                    kl = klo[t]
                    kh = K - kl
                    hd = hd_tiles[t]
                    if t in pre2:
                        g, gv = pre2.pop(t)
                        qn = (t + kl) % 4
                    else:
                        g = gp2.tile([128, max(K, 1) * RW2], bf16, tag="g2")
                        gv = g[:].rearrange("p (k c) -> p k c", c=RW2)
                        qn = t % 4
                        for a in range(0, kl, 8):
                            b = min(a + 8, kl)
                            nc.gpsimd.dma_gather(
                                out_ap=gv[:, a:b, :],
                                in_ap=tab2[0:HB, :],
                                idxs_ap=sgall[:, (koff[t] + a) * 8:(koff[t] + b) * 8],
                                num_idxs=(b - a) * 128,
                                num_idxs_reg=(b - a) * 128,
                                elem_size=RW2,
                                queue_num=qn,
                            )
                            qn = (qn + 1) % 4
                    for a in range(kl, K, 8):
                        b = min(a + 8, K)
                        nc.gpsimd.dma_gather(
                            out_ap=gv[:, a:b, :],
                            in_ap=tab2[HB:NPALL, :],
                            idxs_ap=sgall[:, (koff[t] + a) * 8:(koff[t] + b) * 8],
                            num_idxs=(b - a) * 128,
                            num_idxs_reg=(b - a) * 128,
                            elem_size=RW2,
                            queue_num=qn,
                        )
                        qn = (qn + 1) % 4
                    oh = ohp.tile([128, max(K, 1) * 128], f8, tag="oh")
                    nc.sync.dma_start(
                        out=oh[:],
                        in_=ohd[:, koff[t] * 128:(koff[t] + max(K, 1)) * 128])
                    ps_e = psed.tile([128, max(K, 1)], f32, tag="ed")
                    for j in range(K):
                        psT = pstr8.tile([128, 128], f8, tag="tr8")
                        nc.tensor.transpose(psT[:], oh[:, j * 128:(j + 1) * 128],
                                            ids8[:])
                        ohT = ohtp.tile([128, 128], f8, tag="ohT")
                        nc.scalar.copy(ohT[:], psT[:])
                        nc.tensor.matmul(ps_e[:, j:j + 1], lhsT=ohT[:],
                                         rhs=hd[:, NCLS + 1:NCLS + 2],
                                         start=True, stop=True)
                    return g, gv, oh, ps_e

                def back2(t, st):
                    K = ktile[t]
                    hd = hd_tiles[t]
                    g, gv, oh, ps_e = st
                    if K > 0:
                        ev = wk.tile([128, K], bf16, tag="ev2")
                        nc.vector.tensor_add(ev[:], gv[:, :, NCLS], ps_e[:, 0:K])
                        lr = wk.tile([128, K], bf16, tag="lr2")
                        nc.scalar.activation(lr[:], ev[:], AF.Prelu, alpha=0.2)
                        nc.scalar.activation(gv[:, :, NCLS], lr[:], AF.Exp)
                        gz = gv[:, :, 0:NCLS]
                        exb = gv[:, :, NCLS].unsqueeze(2).to_broadcast(
                            [128, K, NCLS])
                        nc.vector.tensor_mul(gz, gz, exb)
                    ps_f = psp.tile([128, G2W], f32, tag="mm")
                    if K == 0:
                        nc.vector.memset(ps_f[:], 0.0)
                    for j in range(K):
                        nc.tensor.matmul(ps_f[:], lhsT=oh[:, j * 128:(j + 1) * 128],
                                         rhs=g[:, j * RW2:j * RW2 + G2W],
                                         start=(j == 0), stop=(j == K - 1))
                    es = wk.tile([128, 1], bf16, tag="es2")
                    nc.vector.tensor_add(es[:], hd[:, NCLS:NCLS + 1],
                                         hd[:, NCLS + 1:NCLS + 2])
                    lrs = wk.tile([128, 1], bf16, tag="lrs2")
                    nc.scalar.activation(lrs[:], es[:], AF.Prelu, alpha=0.2)
                    exs2 = wk.tile([128, 1], f32, tag="exs2")
                    nc.scalar.activation(exs2[:], lrs[:], AF.Exp)
                    selfc = wk.tile([128, NCLS], bf16, tag="selfc2")
                    nc.vector.tensor_mul(selfc[:], hd[:, 0:NCLS],
                                         exs2[:].to_broadcast([128, NCLS]))
                    den = wk.tile([128, 1], f32, tag="den2")
                    nc.vector.tensor_add(den[:], ps_f[:, NCLS:NCLS + 1], exs2[:])
                    rec = wk.tile([128, 1], f32, tag="rec2")
                    nc.vector.reciprocal(rec[:], den[:])
                    o2n = wk.tile([128, NCLS], f32, tag="o2n")
                    nc.vector.tensor_add(o2n[:], ps_f[:, 0:NCLS], selfc[:])
                    o2t = wk.tile([128, NCLS], bf16, tag="o2b")
                    o2 = o2t[:]
                    nc.vector.tensor_mul(o2, o2n[:],
                                         rec[:].to_broadcast([128, NCLS]))
                    nc.vector.tensor_add(o2, o2, b2s[:])
                    exs = wk.tile([128, NCLS], f32, tag="exs")
                    nc.scalar.activation(exs[:], o2, AF.Exp,
                                         accum_out=ssum[:, t:t + 1])
                    rs = wk.tile([128, 1], f32, tag="rs")
                    nc.vector.reciprocal(rs[:], ssum[:, t:t + 1])
                    nc.scalar.activation(lgs[:, t:t + 1], rs[:], AF.Ln)
                    outf = wk.tile([128, NCLS], f32, tag="outf")
                    nc.scalar.activation(outf[:], o2, AF.Identity,
                                         bias=lgs[:, t:t + 1])
                    nc.sync.dma_start(out=outp[t * 128:(t + 1) * 128, :],
                                      in_=outf[:])

                stf = {}
                for t in range(NT):
                    stf[t] = front2(t)
                    if t >= 1:
                        back2(t - 1, stf.pop(t - 1))
                back2(NT - 1, stf.pop(NT - 1))


    nc.compile()
    return nc


def _get_program(cfg):
    key = tuple(sorted((k, tuple(v) if isinstance(v, tuple) else v)
                       for k, v in cfg.items()))
    if key not in _BUILD_CACHE:
        _BUILD_CACHE[key] = _build_program(**cfg)
    return _BUILD_CACHE[key]


def kernel(**inputs):
    C = 8
    cfg, in_maps, node_at, (N, NCLS) = _host_prep(
        inputs["x"], inputs["edge_index"], inputs["W1"], inputs["a_src1"],
        inputs["a_dst1"], inputs["b1"], inputs["W2"], inputs["a_src2"],
        inputs["a_dst2"], inputs["b2"], C,
    )
    nc = _get_program(cfg)

    from concourse.bass_utils import run_bass_kernel_spmd

    trace = bool(int(os.environ.get("GAT_PROFILE", "0")))
    if trace:
        trace = _register_trace_hook()
    res = run_bass_kernel_spmd(nc, in_maps, list(range(C)), trace=trace)
    if trace and res.exec_time_ns is not None:
        print(f"HW exec time: {res.exec_time_ns} ns", flush=True)
        if res.per_core_scope_times:
            for scope, times in res.per_core_scope_times.items():
                tl = ", ".join(f"{c}:{t/1000:.0f}us" for c, t in sorted(times.items()))
                print(f"  scope {scope}: {tl}", flush=True)
        if res.instructions_and_trace:
            print(f"  trace: {res.instructions_and_trace[1]}", flush=True)
        if res.profile_json:
            print(f"  profile_json: {res.profile_json}", flush=True)

    out = np.empty((N, NCLS), np.float32)
    for c in range(C):
        r = res.results[c]["outp"]
        m = node_at[c] >= 0
        out[node_at[c][m]] = r[m]
    return out


# revision 61
# speedup vs baseline: 1.0302x; 1.0302x over previous
"""Distributed 2-layer GAT on 8 Trainium2 NeuronCores (Bass/Tile).

Strategy (graph/data parallel):
  - Nodes sharded across 8 cores (6250 each, padded to 6272 = 49*128);
    within a core, nodes are sorted by in-degree (desc) into 49 tiles of
    128 so per-tile edge counts match across cores; pads sit in the last
    tile's tail rows.
  - All compute tables in bf16; layer tables ([h | al_src], channel-major
    head layout, rows padded to a 256B multiple for dma_gather) computed
    locally (x @ [W1|W1a_s|W1a_d] extended matmul) and AllGathered so every
    core holds the full node table in DRAM.
  - Edge pass per destination tile: edges (excluding the appended self
    loops) are packed contiguously into 128-slot columns, sorted by source
    row and split into a low/high half-table block (dma_gather indices are
    int16, so each gather addresses < 32768 rows). Per tile: ONE batched
    dma_gather per half fetches all source rows (the 994ns SWDGE fixed
    cost is paid per tile, not per column). Per column: a one-hot
    selection matrix oh[slot, dst] built by a single tensor_scalar
    is_equal against an iota constant; al_dst per edge via PE-transpose of
    oh and a small matmul against the tile's al_dst vector;
    ex = exp(leaky_relu(al_src+al_dst)) overwrites the gathered al_src
    columns so one accumulating matmul per column scatters both the
    ex-weighted rows and the softmax denominators into PSUM.
  - Self loops never touch DRAM: their contribution is computed from the
    resident per-tile h rows and added on the vector engine.
  - Layer-2 local matmul is fused into the layer-1 edge loop so the second
    AllGather starts as soon as the last tile finishes.
"""

import os
import sys
import types

import ml_dtypes
import numpy as np

_BUILD_CACHE = {}
BF = ml_dtypes.bfloat16
F8 = ml_dtypes.float8_e4m3


def _register_trace_hook():
    try:
        if "antenv.axon_hooks" in sys.modules:
            return True
        from trn_agent_boot.trn_boot import _ntff_profile_via_ctypes

        hook = _ntff_profile_via_ctypes("/opt/axon/libaxon_pjrt.so")
        m = types.ModuleType("antenv.axon_hooks")
        m.get_axon_ntff_profile_hook = lambda: hook
        m.set_axon_ntff_profile_hook = lambda h: None
        sys.modules["antenv.axon_hooks"] = m
        return True
    except Exception:
        return False


def _host_prep(x, edge_index, W1, a_src1, a_dst1, b1, W2, a_src2, a_dst2, b2, C):
    x = np.asarray(x, np.float32)
    ei = np.asarray(edge_index)
    W1 = np.asarray(W1, np.float32)
    a_src1 = np.asarray(a_src1, np.float32)
    a_dst1 = np.asarray(a_dst1, np.float32)
    b1 = np.asarray(b1, np.float32)
    W2 = np.asarray(W2, np.float32)
    a_src2 = np.asarray(a_src2, np.float32)
    a_dst2 = np.asarray(a_dst2, np.float32)
    b2 = np.asarray(b2, np.float32)

    N, F = x.shape
    HEADS, HID = a_src1.shape
    D1 = HEADS * HID
    NCLS = W2.shape[1]
    assert N % C == 0
    NPC = N // C
    NT = -(-NPC // 128)
    PADN = NT * 128
    assert PADN > NPC, "need at least one pad slot per core"
    NPALL = C * PADN
    E = ei.shape[1]
    HB = 32768
    RW1 = 384  # [h(256) | al_src(4) | pad] -> 768B rows
    RW2 = 128  # [z2(64) | al_src2(1) | pad] -> 256B rows

    src = ei[0].astype(np.int64)
    dst = ei[1].astype(np.int64)

    # --- per-core in-degree sort into tiles (pads land in last tile) ---
    deg = np.bincount(dst, minlength=N).astype(np.int64)
    pos = np.empty(N, np.int64)
    for c in range(C):
        lo = c * NPC
        order = np.argsort(-deg[lo:lo + NPC], kind="stable")
        p = np.empty(NPC, np.int64)
        p[order] = np.arange(NPC)
        pos[lo:lo + NPC] = p

    ncidx = np.arange(N) // NPC
    node_at = np.full((C, PADN), -1, np.int64)
    node_at[ncidx, pos] = np.arange(N)
    # chunk-major table rows: per chunk, all cores' rows are contiguous.
    # chunk boundary 1 is aligned with the int16 half-table split (HB) so
    # low-half gathers depend only on the first two AllGather chunks.
    NCH = 3
    mid = HB // C
    q1 = (mid // 2 // 128) * 128
    bnds = [0, q1, mid, PADN]
    grow = np.zeros(N, np.int64)
    base = 0
    cbase_h = [0]
    for ci in range(NCH):
        lo, hi = bnds[ci], bnds[ci + 1]
        m = (pos >= lo) & (pos < hi)
        grow[m] = base + ncidx[m] * (hi - lo) + (pos[m] - lo)
        base += C * (hi - lo)
        cbase_h.append(base)
    R1 = bnds[1]

    # --- per-(core, tile) slot packing, sorted by source row, split at HB ---
    ec = dst // NPC
    et = pos[dst] // 128
    gkey = ec * NT + et
    order_e = np.lexsort((grow[src], gkey))
    ks = gkey[order_e]
    s_src = src[order_e]
    s_dst = dst[order_e]
    glo = grow[s_src]
    gstart = np.searchsorted(ks, np.arange(C * NT))
    gend = np.searchsorted(ks, np.arange(C * NT) + 1)
    nlo = np.array([
        np.searchsorted(glo[gstart[g]:gend[g]], HB) for g in range(C * NT)
    ], dtype=np.int64)
    nhi = (gend - gstart) - nlo
    KLO = (-(-nlo.reshape(C, NT) // 128)).max(axis=0)
    KHI = (-(-nhi.reshape(C, NT) // 128)).max(axis=0)
    ktile = KLO + KHI
    koff = np.concatenate([[0], np.cumsum(ktile)]).astype(np.int64)
    SC = int(koff[-1])

    srcg = np.zeros((C, 128, SC), np.int16)  # pad slots read row 0 (oh col is 0)
    rank = np.arange(E) - gstart[ks]
    lo_e = rank < nlo[ks]
    rr = np.where(lo_e, rank, rank - nlo[ks])
    cc_e = ks // NT
    tt_e = ks % NT
    cbase_e = np.where(lo_e, koff[tt_e], koff[tt_e] + KLO[tt_e])
    col_e = cbase_e + rr // 128
    pp_e = rr % 128
    srcg[cc_e, pp_e, col_e] = np.where(lo_e, glo, glo - HB).astype(np.int16)
    dl_e = (pos[s_dst] % 128).astype(np.int64)

    # host-precomputed one-hot selection matrices (slot-major, fp8 — exact
    # 0/1, half the DMA bytes, 2x PE rate):
    #   oh[p, col*128 + q] = 1 iff slot p of col has dest lane q
    # (the dest-major transpose is derived on-chip via PE transpose)
    ohdir = np.zeros((C, 128, SC * 128), F8)
    ohdir[cc_e, pp_e, col_e * 128 + dl_e] = 1

    # wrapped int16 idx layout for dma_gather: within each gather call the
    # i-th index (i = col_local*128 + p) lives at [i % 16, i // 16]; the
    # 16-partition block is replicated across the 8 gpsimd core groups.
    sg = srcg.reshape(C, 8, 16, SC).transpose(0, 2, 3, 1).reshape(C, 16, SC * 8)
    sg16 = np.tile(sg, (1, 8, 1)).copy()

    # --- per-core transposed x shards (pad rows zero), bf16, pre-swizzled so
    # each tile's lhsT loads as one contiguous run per partition ---
    xs = np.zeros((C, PADN, F), np.float32)
    xs[ncidx, pos] = x
    xsT = np.ascontiguousarray(xs.transpose(0, 2, 1)).astype(BF)
    FK_ = F // 128
    xswz = np.ascontiguousarray(
        xsT.reshape(C, FK_, 128, PADN // 128, 128).transpose(0, 2, 3, 1, 4)
    ).reshape(C, 128, (PADN // 128) * FK_ * 128)

    # --- extended weights, channel-major head layout: col j = c*H + h ---
    perm = (np.arange(D1).reshape(HEADS, HID).T).reshape(-1)  # cm -> orig
    W1cm = W1[:, perm]
    Wa_s1 = np.einsum("fhc,hc->fh", W1.reshape(F, HEADS, HID), a_src1)
    Wa_d1 = np.einsum("fhc,hc->fh", W1.reshape(F, HEADS, HID), a_dst1)
    W1e = np.concatenate([W1cm, Wa_s1, Wa_d1], axis=1).astype(BF)
    W2cm = W2[perm, :]
    Wa_s2 = W2 @ a_src2[0]
    Wa_d2 = W2 @ a_dst2[0]
    W2e = np.concatenate(
        [W2cm, Wa_s2[perm, None], Wa_d2[perm, None]], axis=1
    ).astype(BF)
    b1r = np.broadcast_to(b1[perm][None, :], (128, D1)).astype(BF).copy()
    b2r = np.broadcast_to(b2[None, :], (128, NCLS)).astype(np.float32).copy()
    ident = np.eye(128, dtype=np.float32).astype(BF)
    ones = np.ones((128, D1), np.float32).astype(BF)
    ident8 = np.eye(128, dtype=np.float32).astype(F8)
    padneg = np.zeros((128, 1), np.float32)
    lastrow = NPC - (NT - 1) * 128
    padneg[lastrow:, 0] = -15.0

    cfg = dict(F=F, HEADS=HEADS, HID=HID, D1=D1, NCLS=NCLS, NT=NT, PADN=PADN, R1=R1,
               NPC=NPC, NPALL=NPALL, SC=SC, C=C, HB=HB, RW1=RW1, RW2=RW2,
               ktile=tuple(int(k) for k in ktile),
               klo=tuple(int(k) for k in KLO))
    in_maps = []
    for c in range(C):
        in_maps.append({
            "xswz": xswz[c],
            "w1e": W1e,
            "w2e": W2e,
            "b1r": b1r,
            "b2r": b2r,
            "ident": ident,
            "ones": ones,
            "ident8": ident8,
            "sg16": sg16[c],
            "ohdir": ohdir[c],
            "padneg": padneg,
        })
    return cfg, in_maps, node_at, (N, NCLS)


def _build_program(F, HEADS, HID, D1, NCLS, NT, PADN, R1, NPC, NPALL, SC, C, HB,
                   RW1, RW2, ktile, klo):
    import concourse.bacc as bacc
    import concourse.bass as bass
    import concourse.mybir as mybir
    import concourse.tile as tile

    f32 = mybir.dt.float32
    bf16 = mybir.dt.bfloat16
    f8 = mybir.dt.float8e4
    i16 = mybir.dt.int16
    AF = mybir.ActivationFunctionType
    ALU = mybir.AluOpType
    AX = mybir.AxisListType

    H = HEADS
    TW1 = D1 + 2 * H              # [h | al_src | al_dst]
    G1W = D1 + H                  # layer-1 table row payload [h | al_src]
    TW2 = NCLS + 2                # [z2 | al_src2 | al_dst2]
    G2W = NCLS + 1                # layer-2 table row payload [z2 | al_src2]
    FK = F // 128
    DK = D1 // 128
    koff = [0]
    for k in ktile:
        koff.append(koff[-1] + k)

    nc = bacc.Bacc("TRN2", target_bir_lowering=False, debug=False, num_devices=C,
                   num_swdge_queues=4)

    xsw = nc.dram_tensor("xswz", [128, NT * (F // 128) * 128], bf16,
                         kind="ExternalInput")
    w1e = nc.dram_tensor("w1e", [F, TW1], bf16, kind="ExternalInput")
    w2e = nc.dram_tensor("w2e", [D1, TW2], bf16, kind="ExternalInput")
    b1r = nc.dram_tensor("b1r", [128, D1], bf16, kind="ExternalInput")
    b2r = nc.dram_tensor("b2r", [128, NCLS], f32, kind="ExternalInput")
    idn = nc.dram_tensor("ident", [128, 128], bf16, kind="ExternalInput")
    onesd = nc.dram_tensor("ones", [128, D1], bf16, kind="ExternalInput")
    idn8 = nc.dram_tensor("ident8", [128, 128], f8, kind="ExternalInput")
    sgd = nc.dram_tensor("sg16", [128, SC * 8], i16, kind="ExternalInput")
    ohd = nc.dram_tensor("ohdir", [128, SC * 128], f8, kind="ExternalInput")
    pdn = nc.dram_tensor("padneg", [128, 1], f32, kind="ExternalInput")
    outp = nc.dram_tensor("outp", [PADN, NCLS], f32, kind="ExternalOutput")

    loc1 = nc.dram_tensor("loc1", [PADN, RW1], bf16)
    tab1 = nc.dram_tensor("tab1", [NPALL, RW1], bf16, addr_space="Shared")
    loc2 = nc.dram_tensor("loc2", [PADN, RW2], bf16)
    tab2 = nc.dram_tensor("tab2", [NPALL, RW2], bf16, addr_space="Shared")

    rg = [list(range(C))]

    with tile.TileContext(nc) as tc:
        with (
            tc.tile_pool(name="const", bufs=1) as const,
            tc.tile_pool(name="ha", bufs=1) as hap,
            tc.tile_pool(name="hd", bufs=1) as hdp,
            tc.tile_pool(name="wk", bufs=2) as wk,
            tc.tile_pool(name="g", bufs=3) as gp,
            tc.tile_pool(name="g2", bufs=7) as gp2,
            tc.tile_pool(name="oh", bufs=4) as ohp,
            tc.tile_pool(name="ohT", bufs=3) as ohtp,
            tc.tile_pool(name="ps", bufs=2, space="PSUM") as psp,
            tc.tile_pool(name="psmd", bufs=1, space="PSUM") as psmd,
            tc.tile_pool(name="pstr", bufs=1, space="PSUM") as pstr,
            tc.tile_pool(name="pstr8", bufs=2, space="PSUM") as pstr8,
            tc.tile_pool(name="psed", bufs=2, space="PSUM") as psed,
        ):
            # ---- constants ----
            w1t = []
            for kk in range(FK):
                t_ = const.tile([128, TW1], bf16, tag=f"w1_{kk}")
                nc.sync.dma_start(out=t_[:], in_=w1e[kk * 128:(kk + 1) * 128, :])
                w1t.append(t_)
            w2t = []
            for kk in range(DK):
                t_ = const.tile([128, TW2], bf16, tag=f"w2_{kk}")
                nc.sync.dma_start(out=t_[:], in_=w2e[kk * 128:(kk + 1) * 128, :])
                w2t.append(t_)
            b1s = const.tile([128, D1], bf16, tag="b1")
            nc.sync.dma_start(out=b1s[:], in_=b1r[:, :])
            b2s = const.tile([128, NCLS], f32, tag="b2")
            nc.sync.dma_start(out=b2s[:], in_=b2r[:, :])
            ids = const.tile([128, 128], bf16, tag="ident")
            nc.sync.dma_start(out=ids[:], in_=idn[:, :])
            on1 = const.tile([128, D1], bf16, tag="ones")
            nc.sync.dma_start(out=on1[:], in_=onesd[:, :])
            ids8 = const.tile([128, 128], f8, tag="ident8")
            nc.sync.dma_start(out=ids8[:], in_=idn8[:, :])
            sgall = const.tile([128, SC * 8], i16, tag="sgall")
            nc.sync.dma_start(out=sgall[:], in_=sgd[:, :])
            pds = const.tile([128, 1], f32, tag="padneg")
            nc.sync.dma_start(out=pds[:], in_=pdn[:, :])
            ssum = const.tile([128, NT], f32, tag="ssum")
            lgs = const.tile([128, NT], f32, tag="lgs")
            shall = const.tile([128, NT * NCLS], bf16, tag="shall")

            ha_tiles = []
            NCH = 3
            mid = HB // C
            q1 = (mid // 2 // 128) * 128
            bnds = [0, q1, mid, PADN]
            cbase = [0]
            for ci in range(NCH):
                cbase.append(cbase[-1] + C * (bnds[ci + 1] - bnds[ci]))
            # ---- phase A: local h = x @ [W1 | Wa_src | Wa_dst] ----
            with nc.named_scope("l1_local_mm"):
                for t in range(NT):
                    xt = wk.tile([128, FK * 128], bf16, tag="xt")
                    nc.sync.dma_start(
                        out=xt[:],
                        in_=xsw[:, t * FK * 128:(t + 1) * FK * 128])
                    ps_a = psp.tile([128, TW1], f32, tag="mm")
                    for kk in range(FK):
                        nc.tensor.matmul(ps_a[:], lhsT=xt[:, kk * 128:(kk + 1) * 128],
                                         rhs=w1t[kk][:],
                                         start=(kk == 0), stop=(kk == FK - 1))
                    ha = hap.tile([128, TW1], bf16, tag=f"ha_{t}")
                    nc.scalar.copy(ha[:], ps_a[:])
                    if t == NT - 1:
                        nc.vector.tensor_scalar(ha[:, D1:D1 + H],
                                                ha[:, D1:D1 + H],
                                                pds[:, 0:1], None, op0=ALU.add)
                    nc.sync.dma_start(out=loc1[t * 128:(t + 1) * 128, 0:G1W],
                                      in_=ha[:, 0:G1W])
                    ha_tiles.append(ha)
                    for ci in range(NCH - 1):
                        if bnds[ci + 1] > bnds[ci] and t == bnds[ci + 1] // 128 - 1:
                            with nc.named_scope(f"l1_allgather_{ci}"):
                                nc.gpsimd.collective_compute(
                                    "AllGather", mybir.AluOpType.bypass,
                                    replica_groups=rg,
                                    ins=[loc1[bnds[ci]:bnds[ci + 1], :]],
                                    outs=[tab1[cbase[ci]:cbase[ci + 1], :]],
                                )

            # ---- phase B: allgather layer-1 table (tail chunk) ----
            with nc.named_scope("l1_allgather"):
                nc.gpsimd.collective_compute(
                    "AllGather", mybir.AluOpType.bypass, replica_groups=rg,
                    ins=[loc1[bnds[NCH - 1]:PADN, :]],
                    outs=[tab1[cbase[NCH - 1]:NPALL, :]],
                )

            # ---- phase C: layer-1 edge pass (+ fused layer-2 local mm) ----
            hd_tiles = []
            with nc.named_scope("l1_edges"):
                def front1(t):
                    K = ktile[t]
                    kl = klo[t]
                    kh = K - kl
                    ha = ha_tiles[t]
                    g = gp.tile([128, max(K, 1) * RW1], bf16, tag="g1")
                    gv = g[:].rearrange("p (k c) -> p k c", c=RW1)
                    qn = t % 4
                    # the dma_gather ucode crashes above 1024 idxs per call
                    for a in range(0, kl, 8):
                        b = min(a + 8, kl)
                        nc.gpsimd.dma_gather(
                            out_ap=gv[:, a:b, :],
                            in_ap=tab1[0:HB, :],
                            idxs_ap=sgall[:, (koff[t] + a) * 8:(koff[t] + b) * 8],
                            num_idxs=(b - a) * 128,
                            num_idxs_reg=(b - a) * 128,
                            elem_size=RW1,
                            queue_num=qn,
                        )
                        qn = (qn + 1) % 4
                    for a in range(kl, K, 8):
                        b = min(a + 8, K)
                        nc.gpsimd.dma_gather(
                            out_ap=gv[:, a:b, :],
                            in_ap=tab1[HB:NPALL, :],
                            idxs_ap=sgall[:, (koff[t] + a) * 8:(koff[t] + b) * 8],
                            num_idxs=(b - a) * 128,
                            num_idxs_reg=(b - a) * 128,
                            elem_size=RW1,
                            queue_num=qn,
                        )
                        qn = (qn + 1) % 4
                    oh = ohp.tile([128, max(K, 1) * 128], f8, tag="oh")
                    nc.sync.dma_start(
                        out=oh[:],
                        in_=ohd[:, koff[t] * 128:(koff[t] + max(K, 1)) * 128])
                    ps_e = psed.tile([128, max(K, 1) * H], f32, tag="ed")
                    for j in range(K):
                        psT = pstr8.tile([128, 128], f8, tag="tr8")
                        nc.tensor.transpose(psT[:], oh[:, j * 128:(j + 1) * 128],
                                            ids8[:])
                        ohT = ohtp.tile([128, 128], f8, tag="ohT")
                        nc.scalar.copy(ohT[:], psT[:])
                        nc.tensor.matmul(ps_e[:, j * H:(j + 1) * H], lhsT=ohT[:],
                                         rhs=ha[:, D1 + H:D1 + 2 * H],
                                         start=True, stop=True)
                    return g, gv, oh, ps_e

                def back1(t, st):
                    K = ktile[t]
                    ha = ha_tiles[t]
                    g, gv, oh, ps_e = st
                    if K > 0:
                        ev = wk.tile([128, K * H], bf16, tag="ev")
                        nc.vector.tensor_add(
                            ev[:].rearrange("p (k h) -> p k h", h=H),
                            gv[:, :, D1:D1 + H],
                            ps_e[:].rearrange("p (k h) -> p k h", h=H))
                        lr = wk.tile([128, K * H], bf16, tag="lr")
                        nc.scalar.activation(lr[:], ev[:], AF.Prelu, alpha=0.2)
                        nc.scalar.activation(
                            gv[:, :, D1:D1 + H],
                            lr[:].rearrange("p (k h) -> p k h", h=H), AF.Exp)
                        gh = gv[:, :, 0:D1].rearrange("p k (c h) -> p k c h", h=H)
                        exb = gv[:, :, D1:D1 + H].unsqueeze(2).to_broadcast(
                            [128, K, HID, H])
                        nc.vector.tensor_mul(gh, gh, exb)
                    ps_c = psp.tile([128, G1W], f32, tag="mm")
                    if K == 0:
                        nc.vector.memset(ps_c[:], 0.0)
                    for j in range(K):
                        nc.tensor.matmul(ps_c[:], lhsT=oh[:, j * 128:(j + 1) * 128],
                                         rhs=g[:, j * RW1:j * RW1 + G1W],
                                         start=(j == 0), stop=(j == K - 1))
                    es = wk.tile([128, H], bf16, tag="es")
                    nc.vector.tensor_add(es[:], ha[:, D1:D1 + H],
                                         ha[:, D1 + H:D1 + 2 * H])
                    lrs = wk.tile([128, H], bf16, tag="lrs")
                    nc.scalar.activation(lrs[:], es[:], AF.Prelu, alpha=0.2)
                    exs1 = wk.tile([128, H], bf16, tag="exs1")
                    nc.scalar.activation(exs1[:], lrs[:], AF.Exp)
                    selfc = wk.tile([128, D1], bf16, tag="selfc")
                    nc.vector.tensor_mul(
                        selfc[:].rearrange("p (c h) -> p c h", h=H),
                        ha[:, 0:D1].rearrange("p (c h) -> p c h", h=H),
                        exs1[:].unsqueeze(1).to_broadcast([128, HID, H]))
                    den = wk.tile([128, H], f32, tag="den")
                    nc.vector.tensor_add(den[:], ps_c[:, D1:D1 + H], exs1[:])
                    rec = wk.tile([128, H], f32, tag="rec")
                    nc.vector.reciprocal(rec[:], den[:])
                    o1n = wk.tile([128, D1], f32, tag="o1n")
                    nc.vector.tensor_add(o1n[:], ps_c[:, 0:D1], selfc[:])
                    o1 = wk.tile([128, D1], bf16, tag="o1")
                    nc.vector.tensor_mul(
                        o1[:].rearrange("p (c h) -> p c h", h=H),
                        o1n[:].rearrange("p (c h) -> p c h", h=H),
                        rec[:].unsqueeze(1).to_broadcast([128, HID, H]))
                    nc.vector.tensor_add(o1[:], o1[:], b1s[:])
                    rl = wk.tile([128, D1], bf16, tag="rl")
                    nc.scalar.activation(rl[:], o1[:], AF.Relu)
                    tn = wk.tile([128, D1], bf16, tag="tn")
                    nc.vector.tensor_sub(tn[:], o1[:], rl[:])
                    nc.scalar.activation(tn[:], tn[:], AF.Exp)
                    nc.vector.tensor_add(o1[:], rl[:], tn[:])
                    nc.vector.tensor_sub(o1[:], o1[:], on1[:])
                    tts = []
                    for kk in range(DK):
                        ps_t = pstr.tile([128, 128], bf16, tag="tr")
                        nc.tensor.transpose(ps_t[:], o1[:, kk * 128:(kk + 1) * 128],
                                            ids[:])
                        tt = wk.tile([128, 128], bf16, tag=f"tt{kk}")
                        nc.scalar.copy(tt[:], ps_t[:])
                        tts.append(tt)
                    ps_d = psmd.tile([128, TW2], f32, tag="md")
                    for kk in range(DK):
                        nc.tensor.matmul(ps_d[:], lhsT=tts[kk][:], rhs=w2t[kk][:],
                                         start=(kk == 0), stop=(kk == DK - 1))
                    hd = hdp.tile([128, TW2], bf16, tag=f"hd_{t}")
                    nc.scalar.copy(hd[:], ps_d[:])
                    if t == NT - 1:
                        nc.vector.tensor_scalar(hd[:, NCLS:NCLS + 1],
                                                hd[:, NCLS:NCLS + 1],
                                                pds[:, 0:1], None, op0=ALU.add)
                    nc.sync.dma_start(out=loc2[t * 128:(t + 1) * 128, 0:G2W],
                                      in_=hd[:, 0:G2W])
                    hd_tiles.append(hd)
                    for ci in range(NCH - 1):
                        if bnds[ci + 1] > bnds[ci] and t == bnds[ci + 1] // 128 - 1:
                            with nc.named_scope(f"l2_allgather_{ci}"):
                                nc.gpsimd.collective_compute(
                                    "AllGather", mybir.AluOpType.bypass,
                                    replica_groups=rg,
                                    ins=[loc2[bnds[ci]:bnds[ci + 1], :]],
                                    outs=[tab2[cbase[ci]:cbase[ci + 1], :]],
                                )

                # prefetch the first l2 tiles' low-half gathers into the
                # l1 tail where gpsimd is idle (low half of tab2 is complete
                # once the chunk-0 l2 allgather, issued at tile 31, lands).
                pre2 = {}

                def front2lo(t2):
                    K = ktile[t2]
                    kl = klo[t2]
                    g = gp2.tile([128, max(K, 1) * RW2], bf16, tag="g2")
                    gv = g[:].rearrange("p (k c) -> p k c", c=RW2)
                    qn = t2 % 4
                    for a in range(0, kl, 8):
                        b = min(a + 8, kl)
                        nc.gpsimd.dma_gather(
                            out_ap=gv[:, a:b, :],
                            in_ap=tab2[0:HB, :],
                            idxs_ap=sgall[:, (koff[t2] + a) * 8:(koff[t2] + b) * 8],
                            num_idxs=(b - a) * 128,
                            num_idxs_reg=(b - a) * 128,
                            elem_size=RW2,
                            queue_num=qn,
                        )
                        qn = (qn + 1) % 4
                    return g, gv

                NPRE = 6
                stc = {}
                for t in range(NT):
                    stc[t] = front1(t)
                    if t >= 1:
                        back1(t - 1, stc.pop(t - 1))
                        pt = (t - 1) - (NT - 1 - NPRE)
                        if 0 <= pt < NPRE:
                            pre2[pt] = front2lo(pt)
                back1(NT - 1, stc.pop(NT - 1))

            # ---- phase E: allgather layer-2 table (tail chunk) ----
            with nc.named_scope("l2_allgather"):
                nc.gpsimd.collective_compute(
                    "AllGather", mybir.AluOpType.bypass, replica_groups=rg,
                    ins=[loc2[bnds[NCH - 1]:PADN, :]],
                    outs=[tab2[cbase[NCH - 1]:NPALL, :]],
                )

            # ---- phase F: layer-2 edge pass ----
            with nc.named_scope("l2_edges"):
                def front2(t):
                    K = ktile[t]
                    kl = klo[t]
                    kh = K - kl
                    hd = hd_tiles[t]
                    if t in pre2:
                        g, gv = pre2.pop(t)
                        qn = (t + kl) % 4
                    else:
                        g = gp2.tile([128, max(K, 1) * RW2], bf16, tag="g2")
                        gv = g[:].rearrange("p (k c) -> p k c", c=RW2)
                        qn = t % 4
                        for a in range(0, kl, 8):
                            b = min(a + 8, kl)
                            nc.gpsimd.dma_gather(
                                out_ap=gv[:, a:b, :],
                                in_ap=tab2[0:HB, :],
                                idxs_ap=sgall[:, (koff[t] + a) * 8:(koff[t] + b) * 8],
                                num_idxs=(b - a) * 128,
                                num_idxs_reg=(b - a) * 128,
                                elem_size=RW2,
                                queue_num=qn,
                            )
                            qn = (qn + 1) % 4
                    for a in range(kl, K, 8):
                        b = min(a + 8, K)
                        nc.gpsimd.dma_gather(
                            out_ap=gv[:, a:b, :],
                            in_ap=tab2[HB:NPALL, :],
                            idxs_ap=sgall[:, (koff[t] + a) * 8:(koff[t] + b) * 8],
                            num_idxs=(b - a) * 128,
                            num_idxs_reg=(b - a) * 128,
                            elem_size=RW2,
                            queue_num=qn,
                        )
                        qn = (qn + 1) % 4
                    oh = ohp.tile([128, max(K, 1) * 128], f8, tag="oh")
                    nc.sync.dma_start(
                        out=oh[:],
                        in_=ohd[:, koff[t] * 128:(koff[t] + max(K, 1)) * 128])
                    ps_e = psed.tile([128, max(K, 1)], f32, tag="ed")
                    for j in range(K):
                        psT = pstr8.tile([128, 128], f8, tag="tr8")
                        nc.tensor.transpose(psT[:], oh[:, j * 128:(j + 1) * 128],
                                            ids8[:])
                        ohT = ohtp.tile([128, 128], f8, tag="ohT")
                        nc.scalar.copy(ohT[:], psT[:])
                        nc.tensor.matmul(ps_e[:, j:j + 1], lhsT=ohT[:],
                                         rhs=hd[:, NCLS + 1:NCLS + 2],
                                         start=True, stop=True)
                    return g, gv, oh, ps_e

                def back2(t, st):
                    K = ktile[t]
                    hd = hd_tiles[t]
                    g, gv, oh, ps_e = st
                    if K > 0:
                        ev = wk.tile([128, K], bf16, tag="ev2")
                        nc.vector.tensor_add(ev[:], gv[:, :, NCLS], ps_e[:, 0:K])
                        lr = wk.tile([128, K], bf16, tag="lr2")
                        nc.scalar.activation(lr[:], ev[:], AF.Prelu, alpha=0.2)
                        nc.scalar.activation(gv[:, :, NCLS], lr[:], AF.Exp)
                        gz = gv[:, :, 0:NCLS]
                        exb = gv[:, :, NCLS].unsqueeze(2).to_broadcast(
                            [128, K, NCLS])
                        nc.vector.tensor_mul(gz, gz, exb)
                    ps_f = psp.tile([128, G2W], f32, tag="mm")
                    if K == 0:
                        nc.vector.memset(ps_f[:], 0.0)
                    for j in range(K):
                        nc.tensor.matmul(ps_f[:], lhsT=oh[:, j * 128:(j + 1) * 128],
                                         rhs=g[:, j * RW2:j * RW2 + G2W],
                                         start=(j == 0), stop=(j == K - 1))
                    es = wk.tile([128, 1], bf16, tag="es2")
                    nc.vector.tensor_add(es[:], hd[:, NCLS:NCLS + 1],
                                         hd[:, NCLS + 1:NCLS + 2])
                    lrs = wk.tile([128, 1], bf16, tag="lrs2")
                    nc.scalar.activation(lrs[:], es[:], AF.Prelu, alpha=0.2)
                    exs2 = wk.tile([128, 1], f32, tag="exs2")
                    nc.scalar.activation(exs2[:], lrs[:], AF.Exp)
                    selfc = wk.tile([128, NCLS], bf16, tag="selfc2")
                    nc.vector.tensor_mul(selfc[:], hd[:, 0:NCLS],
                                         exs2[:].to_broadcast([128, NCLS]))
                    den = wk.tile([128, 1], f32, tag="den2")
                    nc.vector.tensor_add(den[:], ps_f[:, NCLS:NCLS + 1], exs2[:])
                    rec = wk.tile([128, 1], f32, tag="rec2")
                    nc.vector.reciprocal(rec[:], den[:])
                    o2n = wk.tile([128, NCLS], f32, tag="o2n")
                    nc.vector.tensor_add(o2n[:], ps_f[:, 0:NCLS], selfc[:])
                    o2 = shall[:, t * NCLS:(t + 1) * NCLS]
                    nc.vector.tensor_mul(o2, o2n[:],
                                         rec[:].to_broadcast([128, NCLS]))
                    nc.vector.tensor_add(o2, o2, b2s[:])
                    exs = wk.tile([128, NCLS], f32, tag="exs")
                    nc.scalar.activation(exs[:], o2, AF.Exp,
                                         accum_out=ssum[:, t:t + 1])
                    rs = wk.tile([128, 1], f32, tag="rs")
                    nc.vector.reciprocal(rs[:], ssum[:, t:t + 1])
                    nc.scalar.activation(lgs[:, t:t + 1], rs[:], AF.Ln)
                    outf = wk.tile([128, NCLS], f32, tag="outf")
                    nc.scalar.activation(outf[:], o2, AF.Identity,
                                         bias=lgs[:, t:t + 1])
                    nc.sync.dma_start(out=outp[t * 128:(t + 1) * 128, :],
                                      in_=outf[:])

                stf = {}
                for t in range(NT):
                    stf[t] = front2(t)
                    if t >= 1:
                        back2(t - 1, stf.pop(t - 1))
                back2(NT - 1, stf.pop(NT - 1))


    nc.compile()
    return nc


def _get_program(cfg):
    key = tuple(sorted((k, tuple(v) if isinstance(v, tuple) else v)
                       for k, v in cfg.items()))
    if key not in _BUILD_CACHE:
        _BUILD_CACHE[key] = _build_program(**cfg)
    return _BUILD_CACHE[key]


def kernel(**inputs):
    C = 8
    cfg, in_maps, node_at, (N, NCLS) = _host_prep(
        inputs["x"], inputs["edge_index"], inputs["W1"], inputs["a_src1"],
        inputs["a_dst1"], inputs["b1"], inputs["W2"], inputs["a_src2"],
        inputs["a_dst2"], inputs["b2"], C,
    )
    nc = _get_program(cfg)

    from concourse.bass_utils import run_bass_kernel_spmd

    trace = bool(int(os.environ.get("GAT_PROFILE", "0")))
    if trace:
        trace = _register_trace_hook()
    res = run_bass_kernel_spmd(nc, in_maps, list(range(C)), trace=trace)
    if trace and res.exec_time_ns is not None:
        print(f"HW exec time: {res.exec_time_ns} ns", flush=True)
        if res.per_core_scope_times:
            for scope, times in res.per_core_scope_times.items():
                tl = ", ".join(f"{c}:{t/1000:.0f}us" for c, t in sorted(times.items()))
                print(f"  scope {scope}: {tl}", flush=True)
        if res.instructions_and_trace:
            print(f"  trace: {res.instructions_and_trace[1]}", flush=True)
        if res.profile_json:
            print(f"  profile_json: {res.profile_json}", flush=True)

    out = np.empty((N, NCLS), np.float32)
    for c in range(C):
        r = res.results[c]["outp"]
        m = node_at[c] >= 0
        out[node_at[c][m]] = r[m]
    return out
